# revision 14
# baseline (speedup 1.0000x reference)
"""Max-plus (morphological) dilation 2D on 8 Trainium2 NeuronCores.

out[b,o,y,x] = max_{c,i,j} f[b,c,y+i-2,x+j-2] + h[o,c,i,j]

Strategy
--------
Tensor-parallel over output channels: core k computes o in [4k, 4k+4) for all
8 batch images.  Per core, SBUF partitions hold (c_sub in 16) x (b in 8) = 128
rows; the free axis holds the padded 100x100 image of that (c,b) slice.  Both
kernel shifts (i, j) become free-axis offsets (compute-engine partition
offsets are illegal), and a per-partition scalar vector carries h[o, c(p), i, j].

Per tap (o, i, j, c_block) one of two engine paths accumulates
    acc[p, :] = max(F[p, shifted] + h_vec[p], acc[p, :])
  * DVE path: one fused scalar_tensor_tensor (runs at 1x: ~9.7us)
  * ACT+DVE path: ScalarE activation Identity(in + bias) writes a dense tmp
    (~8.0us, concurrent), then a DVE tensor_tensor max at 2x packed mode
    (~4.9us).
The tap mix is chosen so ScalarE and the DVE finish together.

The channel max-reduce is a log tree: per level an SBUF-SBUF DMA drops the
upper half to partition 0 of a dead tile (TensorTensor needs equal base
partitions), then TT-max combines.  A casting GPSIMD DMA (fp16 -> fp32)
writes DRAM.
"""

import sys

sys.path.insert(0, "/opt/trn_rl_repo")

import numpy as np

B, C, O, H, W, K = 8, 32, 32, 96, 96, 5
N_CORES = 8
O_PER = O // N_CORES          # 4 output channels per core
HP = WP = 100                 # padded image
NPIX = HP * WP                # 10000
NOUT = H * W                  # 9216
NTAP = O_PER * 2 * 25         # hrep columns: (o, c_block, i, j)
PADV = -60000.0               # fp16-safe "-inf": real candidates are ~[-6, 6]

# Per output channel, 13 of 50 taps use the DVE-only path (tensor_scalar add
# at 4x + TT max at 2x); the rest use ACT(add) + DVE TT(max).  This balances
# ScalarE against the DVE (which also carries the channel reduction).
TS_PICK_50 = {(i * 50) // 13 for i in range(13)}

_prog_cache: dict[str, object] = {}


def _build_program():
    import concourse.bacc as bacc
    import concourse.tile as tile
    from concourse import mybir

    FP16 = mybir.dt.float16
    FP32 = mybir.dt.float32
    add, mx = mybir.AluOpType.add, mybir.AluOpType.max
    ident = mybir.ActivationFunctionType.Identity

    nc = bacc.Bacc("TRN2", target_bir_lowering=False, debug=False,
                   num_devices=N_CORES)
    # f pre-transposed on host to [C, B, H, W] so one DMA lands partitions
    # in (c-major, b-minor) order.
    f_dram = nc.dram_tensor("f_t", [C, B, H, W], FP32,
                            kind="ExternalInput").ap()
    hrep_dram = nc.dram_tensor("hrep", [128, NTAP], FP16,
                               kind="ExternalInput").ap()
    hrep32_dram = nc.dram_tensor("hrep32", [128, NTAP], FP32,
                                 kind="ExternalInput").ap()
    out_dram = nc.dram_tensor("out_local", [B, O_PER, H, W], FP32,
                              kind="ExternalOutput").ap()

    QY = 24  # y-rows per load chunk

    with tile.TileContext(nc) as tc:
        with (
            tc.tile_pool(name="main", bufs=1) as pool,
            tc.tile_pool(name="stage", bufs=2) as stage_pool,
            tc.tile_pool(name="tmp", bufs=3) as tmp_pool,
        ):
            Fs = [pool.tile([128, NPIX], FP16, tag=f"F{blk}", name=f"F{blk}")
                  for blk in (0, 1)]
            ACC = [pool.tile([128, NOUT], FP16, tag=f"A{o}", name=f"A{o}")
                   for o in range(O_PER)]
            hrep_sb = pool.tile([128, NTAP], FP16, tag="hrep", name="hrep")
            hrep32_sb = pool.tile([128, NTAP], FP32, tag="hrep32",
                                  name="hrep32")

            nc.sync.dma_start(hrep_sb[:], hrep_dram)
            nc.sync.dma_start(hrep32_sb[:], hrep32_dram)
            for blk in (0, 1):
                nc.gpsimd.memset(Fs[blk][:], PADV)

            # Load fp32 chunks, cast to fp16 into the padded layout (ScalarE).
            for blk in (0, 1):
                for q in range(H // QY):
                    y0 = q * QY
                    st = stage_pool.tile([128, QY * W], FP32, tag="stage",
                                         name=f"st{blk}{q}")
                    nc.sync.dma_start(
                        st[:], f_dram[blk * 16:(blk + 1) * 16, :, y0:y0 + QY, :])
                    st_r = st[:].rearrange("p (y x) -> p y x", x=W)
                    Fr = Fs[blk][:].rearrange("p (y x) -> p y x", x=WP)
                    nc.scalar.copy(Fr[:, 2 + y0:2 + y0 + QY, 2:2 + W], st_r)

            # Main accumulation, o-major: channel o's 50 taps complete early
            # so its reduction overlaps later channels' taps; the scheduler
            # fills DVE gaps with the next channel's work.
            # First tap per o initializes ACC by tensor_copy (no memset, and
            # a 4x copy instead of a 2x TT).
            out_r = out_dram.rearrange("b o y x -> b o (y x)")
            HNF = NOUT // 2  # reduce half-width (fits a dead stage slot)
            for o in range(O_PER):
                k = 0
                for blk in (0, 1):
                    Fr = Fs[blk][:].rearrange("p (y x) -> p y x", x=WP)
                    for ij in range(25):
                        i, j = divmod(ij, 5)
                        in0 = Fr[:, i:i + H, j:j + W]
                        t = (o * 2 + blk) * 25 + ij
                        tmp = tmp_pool.tile([128, NOUT], FP16, tag="tmp",
                                            name=f"tmp{o}_{k}")
                        tmp_r = tmp[:].rearrange("p (y x) -> p y x", x=W)
                        if k in TS_PICK_50:
                            nc.vector.tensor_scalar(
                                tmp_r[:, :, :], in0, hrep32_sb[:, t:t + 1],
                                None, op0=add)
                        else:
                            nc.scalar.activation(tmp_r[:, :, :], in0, ident,
                                                 bias=hrep_sb[:, t:t + 1],
                                                 scale=1.0)
                        if k == 0:
                            nc.vector.tensor_copy(ACC[o][:, :], tmp[:, :])
                        else:
                            nc.vector.tensor_tensor(ACC[o][:, :], ACC[o][:, :],
                                                    tmp[:, :], op=mx)
                        k += 1
                # Channel reduction for this o: 16 (c-major) groups of 8 -> 1.
                # TensorTensor requires both SBUF inputs at the same base
                # partition, so each level first DMAs the upper partitions
                # down to partition 0 of scratch (reusing dead stage-pool
                # slots, half the free dim at a time), then TT-maxes.
                a = ACC[o]
                for n in (64, 32, 16, 8):
                    for hf in (0, 1):
                        s = stage_pool.tile([64, HNF], FP16, tag="stage",
                                            name=f"scr{o}_{n}_{hf}")
                        nc.sync.dma_start(
                            s[0:n, :], a[n:2 * n, hf * HNF:(hf + 1) * HNF])
                        nc.vector.tensor_tensor(
                            a[0:n, hf * HNF:(hf + 1) * HNF],
                            a[0:n, hf * HNF:(hf + 1) * HNF],
                            s[0:n, :], op=mx)
                # SWDGE casting DMA fp16 -> fp32 straight to DRAM.
                nc.gpsimd.dma_start(out_r[:, o, :], a[0:8, :])

    nc.compile()
    return nc


def _get_program():
    if "nc" not in _prog_cache:
        _prog_cache["nc"] = _build_program()
    return _prog_cache["nc"]


def _make_in_maps(f: np.ndarray, h: np.ndarray):
    f_t = np.ascontiguousarray(f.transpose(1, 0, 2, 3)).astype(np.float32)
    in_maps = []
    for core in range(N_CORES):
        h_core = h[core * O_PER:(core + 1) * O_PER]  # [4, 32, 5, 5]
        hrep = np.empty((128, NTAP), np.float16)
        for o in range(O_PER):
            for blk in (0, 1):
                sub = h_core[o, blk * 16:(blk + 1) * 16]     # [16, 5, 5]
                cols = sub.reshape(16, 25)                   # [c_sub, ij]
                t0 = (o * 2 + blk) * 25
                hrep[:, t0:t0 + 25] = np.repeat(
                    cols.astype(np.float16), 8, axis=0)      # p = c*8 + b
        in_maps.append({"f_t": f_t, "hrep": hrep,
                        "hrep32": hrep.astype(np.float32)})
    return in_maps


def kernel(f: np.ndarray, h: np.ndarray, _trace: bool = False):
    from concourse.bass_utils import run_bass_kernel_spmd

    nc = _get_program()
    in_maps = _make_in_maps(np.asarray(f), np.asarray(h))
    res = run_bass_kernel_spmd(nc, in_maps, list(range(N_CORES)),
                               trace=_trace)
    out = np.empty((B, O, H, W), np.float32)
    for core in range(N_CORES):
        out[:, core * O_PER:(core + 1) * O_PER] = res.results[core]["out_local"]
    if _trace:
        return out, res
    return out


# revision 17
# speedup vs baseline: 1.0093x; 1.0093x over previous
"""Max-plus (morphological) dilation 2D on 8 Trainium2 NeuronCores.

out[b,o,y,x] = max_{c,i,j} f[b,c,y+i-2,x+j-2] + h[o,c,i,j]

Strategy
--------
Tensor-parallel over output channels: core k computes o in [4k, 4k+4) for all
8 batch images.  Per core, SBUF partitions hold (c_sub in 16) x (b in 8) = 128
rows; the free axis holds the padded 100x100 image of that (c,b) slice.  Both
kernel shifts (i, j) become free-axis offsets (compute-engine partition
offsets are illegal), and a per-partition scalar vector carries h[o, c(p), i, j].

Per tap (o, i, j, c_block) one of two engine paths accumulates
    acc[p, :] = max(F[p, shifted] + h_vec[p], acc[p, :])
  * DVE path: one fused scalar_tensor_tensor (runs at 1x: ~9.7us)
  * ACT+DVE path: ScalarE activation Identity(in + bias) writes a dense tmp
    (~8.0us, concurrent), then a DVE tensor_tensor max at 2x packed mode
    (~4.9us).
The tap mix is chosen so ScalarE and the DVE finish together.

The channel max-reduce is a log tree: per level an SBUF-SBUF DMA drops the
upper half to partition 0 of a dead tile (TensorTensor needs equal base
partitions), then TT-max combines.  A casting GPSIMD DMA (fp16 -> fp32)
writes DRAM.
"""

import sys

sys.path.insert(0, "/opt/trn_rl_repo")

import numpy as np

B, C, O, H, W, K = 8, 32, 32, 96, 96, 5
N_CORES = 8
O_PER = O // N_CORES          # 4 output channels per core
HP = WP = 100                 # padded image
NPIX = HP * WP                # 10000
NOUT = H * W                  # 9216
NTAP = O_PER * 2 * 25         # hrep columns: (o, c_block, i, j)
PADV = -60000.0               # fp16-safe "-inf": real candidates are ~[-6, 6]

# Of each phase's 100 taps (two o-chains interleaved), this many use the
# DVE-only path (tensor_scalar add at 4x + TT max at 2x); the rest use
# ACT(add) + DVE TT(max).  Phase B carries phase A's channel reduction on
# the DVE, so it shifts a few taps back to ACT.
TS_COUNT = (29, 26)

_prog_cache: dict[str, object] = {}


def _build_program():
    import concourse.bacc as bacc
    import concourse.tile as tile
    from concourse import mybir

    FP16 = mybir.dt.float16
    FP32 = mybir.dt.float32
    add, mx = mybir.AluOpType.add, mybir.AluOpType.max
    ident = mybir.ActivationFunctionType.Identity

    nc = bacc.Bacc("TRN2", target_bir_lowering=False, debug=False,
                   num_devices=N_CORES)
    # f pre-transposed on host to [C, B, H, W] so one DMA lands partitions
    # in (c-major, b-minor) order.
    f_dram = nc.dram_tensor("f_t", [C, B, H, W], FP32,
                            kind="ExternalInput").ap()
    hrep_dram = nc.dram_tensor("hrep", [128, NTAP], FP16,
                               kind="ExternalInput").ap()
    hrep32_dram = nc.dram_tensor("hrep32", [128, NTAP], FP32,
                                 kind="ExternalInput").ap()
    out_dram = nc.dram_tensor("out_local", [B, O_PER, H, W], FP32,
                              kind="ExternalOutput").ap()

    QY = 24  # y-rows per load chunk

    with tile.TileContext(nc) as tc:
        with (
            tc.tile_pool(name="main", bufs=1) as pool,
            tc.tile_pool(name="stage", bufs=2) as stage_pool,
            tc.tile_pool(name="tmp", bufs=3) as tmp_pool,
        ):
            Fs = [pool.tile([128, NPIX], FP16, tag=f"F{blk}", name=f"F{blk}")
                  for blk in (0, 1)]
            ACC = [pool.tile([128, NOUT], FP16, tag=f"A{o}", name=f"A{o}")
                   for o in range(O_PER)]
            hrep_sb = pool.tile([128, NTAP], FP16, tag="hrep", name="hrep")
            hrep32_sb = pool.tile([128, NTAP], FP32, tag="hrep32",
                                  name="hrep32")

            nc.sync.dma_start(hrep_sb[:], hrep_dram)
            nc.sync.dma_start(hrep32_sb[:], hrep32_dram)
            for blk in (0, 1):
                nc.gpsimd.memset(Fs[blk][:], PADV)

            # Load fp32 chunks, cast to fp16 into the padded layout (ScalarE).
            for blk in (0, 1):
                for q in range(H // QY):
                    y0 = q * QY
                    st = stage_pool.tile([128, QY * W], FP32, tag="stage",
                                         name=f"st{blk}{q}")
                    nc.sync.dma_start(
                        st[:], f_dram[blk * 16:(blk + 1) * 16, :, y0:y0 + QY, :])
                    st_r = st[:].rearrange("p (y x) -> p y x", x=W)
                    Fr = Fs[blk][:].rearrange("p (y x) -> p y x", x=WP)
                    nc.scalar.copy(Fr[:, 2 + y0:2 + y0 + QY, 2:2 + W], st_r)

            # Main accumulation in two phases of two o-chains each.  The
            # first tap per o initializes ACC by tensor_copy (4x, no memset).
            out_r = out_dram.rearrange("b o y x -> b o (y x)")
            HNF = NOUT // 2  # reduce half-width (fits a dead stage slot)

            def reduce_o(o):
                # Channel reduction: 16 (c-major) partition groups of 8 -> 1.
                # TensorTensor requires both SBUF inputs at the same base
                # partition, so each level first DMAs the upper partitions
                # down to partition 0 of scratch (dead stage-pool slots,
                # half the free dim at a time), then TT-maxes.
                a = ACC[o]
                for n in (64, 32, 16, 8):
                    for hf in (0, 1):
                        s = stage_pool.tile([64, HNF], FP16, tag="stage",
                                            name=f"scr{o}_{n}_{hf}")
                        nc.sync.dma_start(
                            s[0:n, :], a[n:2 * n, hf * HNF:(hf + 1) * HNF])
                        nc.vector.tensor_tensor(
                            a[0:n, hf * HNF:(hf + 1) * HNF],
                            a[0:n, hf * HNF:(hf + 1) * HNF],
                            s[0:n, :], op=mx)
                # SWDGE casting DMA fp16 -> fp32 straight to DRAM.
                nc.gpsimd.dma_start(out_r[:, o, :], a[0:8, :])

            for phase in (0, 1):
                o_pair = (2 * phase, 2 * phase + 1)
                n_ts = TS_COUNT[phase]
                ts_marks = {(i * 100) // n_ts for i in range(n_ts)}
                k = 0
                for blk in (0, 1):
                    Fr = Fs[blk][:].rearrange("p (y x) -> p y x", x=WP)
                    for ij in range(25):
                        i, j = divmod(ij, 5)
                        in0 = Fr[:, i:i + H, j:j + W]
                        for o in o_pair:
                            t = (o * 2 + blk) * 25 + ij
                            tmp = tmp_pool.tile([128, NOUT], FP16, tag="tmp",
                                                name=f"tmp{o}_{k}")
                            tmp_r = tmp[:].rearrange("p (y x) -> p y x", x=W)
                            if k in ts_marks:
                                nc.vector.tensor_scalar(
                                    tmp_r[:, :, :], in0,
                                    hrep32_sb[:, t:t + 1], None, op0=add)
                            else:
                                nc.scalar.activation(
                                    tmp_r[:, :, :], in0, ident,
                                    bias=hrep_sb[:, t:t + 1], scale=1.0)
                            if k < 2:
                                nc.vector.tensor_copy(ACC[o][:, :], tmp[:, :])
                            else:
                                nc.vector.tensor_tensor(
                                    ACC[o][:, :], ACC[o][:, :], tmp[:, :],
                                    op=mx)
                            k += 1
                        # a few taps into phase B, slot in phase A's
                        # reductions so their DMAs overlap tap compute
                        if phase == 1 and blk == 0 and ij == 3 and o == o_pair[1]:
                            reduce_o(0)
                            reduce_o(1)
            reduce_o(2)
            reduce_o(3)

    nc.compile()
    return nc


def _get_program():
    if "nc" not in _prog_cache:
        _prog_cache["nc"] = _build_program()
    return _prog_cache["nc"]


def _make_in_maps(f: np.ndarray, h: np.ndarray):
    f_t = np.ascontiguousarray(f.transpose(1, 0, 2, 3)).astype(np.float32)
    in_maps = []
    for core in range(N_CORES):
        h_core = h[core * O_PER:(core + 1) * O_PER]  # [4, 32, 5, 5]
        hrep = np.empty((128, NTAP), np.float16)
        for o in range(O_PER):
            for blk in (0, 1):
                sub = h_core[o, blk * 16:(blk + 1) * 16]     # [16, 5, 5]
                cols = sub.reshape(16, 25)                   # [c_sub, ij]
                t0 = (o * 2 + blk) * 25
                hrep[:, t0:t0 + 25] = np.repeat(
                    cols.astype(np.float16), 8, axis=0)      # p = c*8 + b
        in_maps.append({"f_t": f_t, "hrep": hrep,
                        "hrep32": hrep.astype(np.float32)})
    return in_maps


def kernel(f: np.ndarray, h: np.ndarray, _trace: bool = False):
    from concourse.bass_utils import run_bass_kernel_spmd

    nc = _get_program()
    in_maps = _make_in_maps(np.asarray(f), np.asarray(h))
    res = run_bass_kernel_spmd(nc, in_maps, list(range(N_CORES)),
                               trace=_trace)
    out = np.empty((B, O, H, W), np.float32)
    for core in range(N_CORES):
        out[:, core * O_PER:(core + 1) * O_PER] = res.results[core]["out_local"]
    if _trace:
        return out, res
    return out
